# revision 3
# baseline (speedup 1.0000x reference)
"""Trainium2 Bass kernel for nn_CholeskyResHead_68255620268805.

Reference math (per mixture component c of C=10):
    Ks = Ls @ Ls.T ; Kt = Lt @ Lt.T            (spatial 207x207, temporal 12x12)
    M  = (Ks  (x)  Kt + sig^2 I)^-1            (via eigh + explicit kron in ref)
    quad[b,c] = r_b^T M r_b                    (r = (target-mu).reshape(b, n*t))
    ll = -0.5*n*t*log(2pi) - 0.5*quad + n*Vlog + t*Ulog + log w
    nll_loss = mean_b(-logsumexp_c ll)
    mse_loss = mean(|mu-target| * mask/mean(mask)),  mask = (unscaled != 0)
    out = 0.1*nll_loss + 0.9*mse_loss

Key identity used here: with Ks = Us Ds Us^T, Kt = Ut Dt Ut^T,
    quad[b,c] = sum_{m,j} (Us^T R_b Ut)[m,j]^2 / (Ds[m] Dt[j] + sig^2)
so the (nt x nt) kron inverse never needs to be materialized.

Distribution: data-parallel over batch (64 -> 8 per core), all 10 components
on every core; no collectives.  Host does the small eigendecompositions
(parameter prep, invariant for the quadratic form) and the final 8-way
scalar combine; the device does everything batch-sized: residuals, the two
GEMM stages, the capacitance-weighted square-reduce, quad assembly, the
log-sum-exp, and the masked-MAE partial sums.
"""

import numpy as np

B, N, T, C = 64, 207, 12, 10
NT = N * T
RHO = 0.1
LOG2PI = float(np.log(2.0 * np.pi))
NCORES = 8
BL = B // NCORES          # local batches per core
BT = BL * T               # 96 = (b, t) pairs per core
P0 = 128                  # first spatial chunk (partition dim)
P1 = N - P0               # 79

_CACHE: dict = {}


def _declare_io(nc, f32):
    t = {}
    t["mu_t"] = nc.dram_tensor("mu_t", [N, BT], f32, kind="ExternalInput")
    t["tg_t"] = nc.dram_tensor("tg_t", [N, BT], f32, kind="ExternalInput")
    t["un_t"] = nc.dram_tensor("un_t", [N, BT], f32, kind="ExternalInput")
    t["us"] = nc.dram_tensor("us", [C, N, N], f32, kind="ExternalInput")
    t["wblk"] = nc.dram_tensor("wblk", [C, BT, BT], f32, kind="ExternalInput")
    t["icr"] = nc.dram_tensor("icr", [C, BT, N], f32, kind="ExternalInput")
    t["em"] = nc.dram_tensor("em", [BT, BL], f32, kind="ExternalInput")
    t["onesv"] = nc.dram_tensor("onesv", [P0, 1], f32, kind="ExternalInput")
    t["m2"] = nc.dram_tensor("m2", [BL, C], f32, kind="ExternalInput")
    t["out_part"] = nc.dram_tensor("out_part", [1, 3], f32, kind="ExternalOutput")
    t["quad_out"] = nc.dram_tensor("quad_out", [BL, C], f32, kind="ExternalOutput")
    return t


def _emit_body(nc, tc, io):
    import concourse.mybir as mybir

    f32 = mybir.dt.float32
    AF = mybir.ActivationFunctionType
    OP = mybir.AluOpType
    AX = mybir.AxisListType

    mu_t, tg_t, un_t = io["mu_t"], io["tg_t"], io["un_t"]
    us, wblk, icr = io["us"], io["wblk"], io["icr"]
    em, onesv, m2 = io["em"], io["onesv"], io["m2"]
    out_part, quad_out = io["out_part"], io["quad_out"]

    with (
        tc.tile_pool(name="cst", bufs=1) as cst,
        tc.tile_pool(name="usp", bufs=3) as usp,
        tc.tile_pool(name="wkp", bufs=3) as wkp,
        tc.tile_pool(name="icp", bufs=3) as icp,
        tc.tile_pool(name="ztp", bufs=3) as ztp,
        tc.tile_pool(name="sqp", bufs=3) as sqp,
        tc.tile_pool(name="scp", bufs=2) as scp,
        tc.tile_pool(name="ps_z", bufs=2, space="PSUM") as ps_z,
        tc.tile_pool(name="ps_y", bufs=2, space="PSUM") as ps_y,
        tc.tile_pool(name="ps_s", bufs=1, space="PSUM") as ps_s,
    ):
        # ---- load inputs ----
        mu0 = cst.tile([P0, BT], f32, tag="mu0")
        mu1 = cst.tile([P1, BT], f32, tag="mu1")
        tg0 = cst.tile([P0, BT], f32, tag="tg0")
        tg1 = cst.tile([P1, BT], f32, tag="tg1")
        un0 = cst.tile([P0, BT], f32, tag="un0")
        un1 = cst.tile([P1, BT], f32, tag="un1")
        nc.sync.dma_start(mu0[:], mu_t[0:P0, :])
        nc.sync.dma_start(mu1[:], mu_t[P0:N, :])
        nc.sync.dma_start(tg0[:], tg_t[0:P0, :])
        nc.sync.dma_start(tg1[:], tg_t[P0:N, :])
        nc.sync.dma_start(un0[:], un_t[0:P0, :])
        nc.sync.dma_start(un1[:], un_t[P0:N, :])

        emt = cst.tile([BT, BL], f32, tag="emt")
        nc.sync.dma_start(emt[:], em[:])
        onest = cst.tile([P0, 1], f32, tag="onest")
        nc.sync.dma_start(onest[:], onesv[:])
        m2t = cst.tile([BL, C], f32, tag="m2t")
        nc.sync.dma_start(m2t[:], m2[:])

        # ---- residuals (also the matmul lhsT) ----
        r0 = cst.tile([P0, BT], f32, tag="r0")
        r1 = cst.tile([P1, BT], f32, tag="r1")
        nc.vector.tensor_sub(r0[:], tg0[:], mu0[:])
        nc.vector.tensor_sub(r1[:], tg1[:], mu1[:])

        # ---- masked-MAE partial sums ----
        mk0 = cst.tile([P0, BT], f32, tag="mk0")
        mk1 = cst.tile([P1, BT], f32, tag="mk1")
        nc.vector.tensor_scalar(mk0[:], un0[:], 0.0, None, op0=OP.not_equal)
        nc.vector.tensor_scalar(mk1[:], un1[:], 0.0, None, op0=OP.not_equal)
        mr0 = cst.tile([P0, BT], f32, tag="mr0")
        mr1 = cst.tile([P1, BT], f32, tag="mr1")
        nc.vector.tensor_mul(mr0[:], r0[:], mk0[:])
        nc.vector.tensor_mul(mr1[:], r1[:], mk1[:])
        pt0 = cst.tile([P0, 2], f32, tag="pt0")
        pt1 = cst.tile([P1, 2], f32, tag="pt1")
        nc.vector.tensor_reduce(
            pt0[:, 0:1], mr0[:], axis=AX.X, op=OP.add, apply_absolute_value=True
        )
        nc.vector.tensor_reduce(pt0[:, 1:2], mk0[:], axis=AX.X, op=OP.add)
        nc.vector.tensor_reduce(
            pt1[:, 0:1], mr1[:], axis=AX.X, op=OP.add, apply_absolute_value=True
        )
        nc.vector.tensor_reduce(pt1[:, 1:2], mk1[:], axis=AX.X, op=OP.add)
        mae_ps = ps_s.tile([1, 2], f32, tag="mae_ps")
        nc.tensor.matmul(mae_ps[:], onest[:], pt0[:], start=True, stop=False)
        nc.tensor.matmul(mae_ps[:], onest[0:P1, :], pt1[:], start=False, stop=True)

        # ---- per-component quadratic forms ----
        # S[(b,j), c] = sum_m (Us_c^T R_b Ut_c)[m,j]^2 * icap_c[j,m]
        S = cst.tile([BT, C], f32, tag="S")
        for c in range(C):
            us0 = usp.tile([P0, N], f32, tag="us0")
            us1 = usp.tile([P1, N], f32, tag="us1")
            nc.sync.dma_start(us0[:], us[c, 0:P0, :])
            nc.sync.dma_start(us1[:], us[c, P0:N, :])
            wk = wkp.tile([BT, BT], f32, tag="wk")
            nc.sync.dma_start(wk[:], wblk[c, :, :])
            ic = icp.tile([BT, N], f32, tag="ic")
            nc.sync.dma_start(ic[:], icr[c, :, :])

            # Z^T[(b,t), m] = sum_n resid[n,(b,t)] Us[n,m]
            zt = ps_z.tile([BT, N], f32, tag="zt")
            nc.tensor.matmul(zt[:], r0[:], us0[:], start=True, stop=False)
            nc.tensor.matmul(zt[:], r1[:], us1[:], start=False, stop=True)
            ztsb = ztp.tile([BT, N], f32, tag="ztsb")
            nc.scalar.copy(ztsb[:], zt[:])

            # Yt[(b,j), m] = sum_t Ut[t,j] Z^T[(b,t), m]
            yt = ps_y.tile([BT, N], f32, tag="yt")
            nc.tensor.matmul(yt[:], wk[:], ztsb[:], start=True, stop=True)

            sq = sqp.tile([BT, N], f32, tag="sq")
            nc.scalar.activation(sq[:], yt[:], AF.Square)
            scr = scp.tile([BT, N], f32, tag="scr")
            # scr = (sq * 1.0) * ic ; S[:,c] = sum_m scr
            # (InstTensorTensorReduce crashes the runtime on this stack;
            # scalar_tensor_tensor with accum_out is the working
            # one-instruction equivalent.)
            nc.vector.scalar_tensor_tensor(
                scr[:],
                sq[:],
                1.0,
                ic[:],
                op0=OP.mult,
                op1=OP.mult,
                accum_out=S[:, c : c + 1],
            )

        # ---- quad[b, c] = sum_j S[(b,j), c] ----
        q_ps = ps_s.tile([BL, C], f32, tag="q_ps")
        nc.tensor.matmul(q_ps[:], emt[:], S[:], start=True, stop=True)
        q_sb = cst.tile([BL, C], f32, tag="q_sb")
        nc.scalar.copy(q_sb[:], q_ps[:])
        nc.sync.dma_start(quad_out[:], q_sb[:])

        # ---- ll + logsumexp over components ----
        ll = cst.tile([BL, C], f32, tag="ll")
        nc.vector.scalar_tensor_tensor(
            ll[:], q_ps[:], -0.5, m2t[:], op0=OP.mult, op1=OP.add
        )
        negmx = cst.tile([BL, 1], f32, tag="negmx")
        nc.vector.tensor_reduce(negmx[:], ll[:], axis=AX.X, op=OP.max, negate=True)
        ex = cst.tile([BL, C], f32, tag="ex")
        nc.scalar.activation(ex[:], ll[:], AF.Exp, bias=negmx[:, 0:1])
        se = cst.tile([BL, 1], f32, tag="se")
        nc.vector.tensor_reduce(se[:], ex[:], axis=AX.X, op=OP.add)
        lse = cst.tile([BL, 1], f32, tag="lse")
        nc.scalar.activation(lse[:], se[:], AF.Ln)
        v = cst.tile([BL, 1], f32, tag="v")
        nc.vector.tensor_sub(v[:], lse[:], negmx[:])  # = logsumexp_c ll

        sv_ps = ps_s.tile([1, 1], f32, tag="sv_ps")
        nc.tensor.matmul(sv_ps[:], onest[0:BL, :], v[:], start=True, stop=True)

        # ---- pack per-core partials: [sum_b lse, sum |r|*mask, sum mask] ----
        part = cst.tile([1, 3], f32, tag="part")
        nc.scalar.copy(part[0:1, 0:1], sv_ps[:])
        nc.scalar.copy(part[0:1, 1:3], mae_ps[:])
        nc.sync.dma_start(out_part[:], part[:])


def _build_program():
    import concourse.bacc as bacc
    import concourse.mybir as mybir
    from concourse import tile

    f32 = mybir.dt.float32
    nc = bacc.Bacc(None, target_bir_lowering=False)
    io = _declare_io(nc, f32)
    with tile.TileContext(nc) as tc:
        _emit_body(nc, tc, io)
    nc.compile()
    return nc


def _get_program():
    if "nc" not in _CACHE:
        _CACHE["nc"] = _build_program()
    return _CACHE["nc"]


def _host_prep(mu, target, unscaled_target, w, sigma, L_spatial, L_temporal):
    """Per-core input maps. Heavy lifting (eigh of the small covariance
    factors) in float64 for accuracy; everything shipped as float32."""
    f = np.float32
    mu = np.asarray(mu, dtype=f)
    target = np.asarray(target, dtype=f)
    unscaled_target = np.asarray(unscaled_target, dtype=f)
    w = np.asarray(w, dtype=f)
    Ls = np.asarray(L_spatial, dtype=np.float64)
    Lt = np.asarray(L_temporal, dtype=np.float64)

    Ks = Ls @ np.transpose(Ls, (0, 2, 1))        # (C, N, N)
    Kt = Lt @ np.transpose(Lt, (0, 2, 1))        # (C, T, T)
    Ds, Us = np.linalg.eigh(Ks)                   # (C, N), (C, N, N)
    Dt, Ut = np.linalg.eigh(Kt)                   # (C, T), (C, T, T)
    sig2 = np.asarray(sigma, dtype=np.float64) ** 2

    # icap[c, j, m] = 1 / (Ds[c, m] * Dt[c, j] + sig2[c])
    icap = 1.0 / (Dt[:, :, None] * Ds[:, None, :] + sig2[:, None, None])
    icr = np.tile(icap, (1, BL, 1)).astype(f)     # (C, BT, N)
    wblk = np.stack([np.kron(np.eye(BL), Ut[c]) for c in range(C)]).astype(f)
    us32 = np.ascontiguousarray(Us.astype(f))

    em = np.kron(np.eye(BL, dtype=f), np.ones((T, 1), dtype=f))  # (BT, BL)
    ones = np.ones((P0, 1), dtype=f)

    Ulog = np.sum(np.log(np.einsum("cii->ci", Ls)), axis=1)       # spatial
    Vlog = np.sum(np.log(np.einsum("cii->ci", Lt)), axis=1)       # temporal
    logw = np.log(np.asarray(w, dtype=np.float64)[..., 0])        # (B, C)
    m2_full = (
        -0.5 * NT * LOG2PI + N * Vlog[None, :] + T * Ulog[None, :] + logw
    ).astype(f)                                                    # (B, C)

    in_maps = []
    for k in range(NCORES):
        sl = slice(k * BL, (k + 1) * BL)
        tr = lambda x: np.ascontiguousarray(
            x[sl].transpose(1, 0, 2).reshape(N, BT)
        )
        in_maps.append(
            {
                "mu_t": tr(mu),
                "tg_t": tr(target),
                "un_t": tr(unscaled_target),
                "us": us32,
                "wblk": wblk,
                "icr": icr,
                "em": em,
                "onesv": ones,
                "m2": np.ascontiguousarray(m2_full[sl]),
            }
        )
    return in_maps


def kernel(**inputs) -> np.ndarray:
    from concourse.bass_utils import run_bass_kernel_spmd

    nc = _get_program()
    in_maps = _host_prep(
        inputs["mu"],
        inputs["target"],
        inputs["unscaled_target"],
        inputs["w"],
        inputs["sigma"],
        inputs["L_spatial"],
        inputs["L_temporal"],
    )
    res = run_bass_kernel_spmd(nc, in_maps, list(range(NCORES))).results

    sum_lse = 0.0
    sum_abs = 0.0
    sum_msk = 0.0
    for k in range(NCORES):
        p = res[k]["out_part"]
        sum_lse += float(p[0, 0])
        sum_abs += float(p[0, 1])
        sum_msk += float(p[0, 2])
    nll_loss = -(np.float32(sum_lse) / np.float32(B))
    mse_loss = np.float32(sum_abs) / np.float32(sum_msk)
    out = np.float32(RHO) * nll_loss + np.float32(1.0 - RHO) * mse_loss
    return np.asarray(out, dtype=np.float32)
